# revision 9
# baseline (speedup 1.0000x reference)
"""Exponential smoothing (per-channel EMA over time) on 8 Trainium2 cores.

  s_0 = x_0 ; s_t = a * x_t + (1 - a) * s_{t-1},  a = sigmoid(alpha)  (per channel)

Full shapes: x (16, 4096, 512) f32, alpha (1, 1, 512) f32 -> out (16, 4096, 512).
Sharding: data-parallel over batch B (16 -> 2 per core); alpha replicated.

Per core, per 1024-step time chunk:
  1. DMA-loads x (cast to bf16 on host) in native layout (t on partitions).
  2. Transposes 128x128 blocks on the tensor engine into 2-bank PSUM tiles
     (time on the free axis, channels on partitions).
  3. Runs a hand-built custom DVE op (EMA_PAGED_ANT, registered below) that
     scans r_t = w*r_{t-1} + x_t directly out of PSUM at ~1.1 cyc/element
     (vs ~2.3 for the stock TensorTensorScanArith), writing bf16 r to SBUF.
     The scan is the rescaled form r = s/a, so no pre-scale pass is needed;
     chunk chaining passes the previous chunk's last column as the carry.
     Chunk 0 seeds with r_{-1} = x_0/a, making s_0 = x_0 exactly.
  4. Transposes back via a regular matmul against diag(a) (contracting the
     channel partitions), so s = a*r is applied by the tensor engine for free.
  5. Scalar engine evacuates PSUM -> bf16 SBUF, GpSimd SWDGE stores to HBM.
Host converts the bf16 y back to f32.
"""

from contextlib import ExitStack

import ml_dtypes
import numpy as np

import concourse.bass as bass
import concourse.dve_ops as dve_ops
import concourse.tile as tile
from concourse import bacc, mybir
from concourse.bass_utils import run_bass_kernel_spmd
from concourse.dve_ops import DveOp
from concourse.dve_spec import (
    C0,
    C1,
    AluOp,
    Bin,
    One,
    Spec,
    Src0,
    Src1,
    _Placement,
    _Stage,
    _State,
    _assemble,
    COUNT_ONCE,
    PREV,
)
from concourse.dve_uop import AluInp, DveOpSpec, OutSel, Trigger
from concourse.masks import make_identity

# ---------------------------------------------------------------------------
# Custom DVE op: paged EMA scan, r_k = w*r_{k-1} + u_k at ~1.1 cyc/element.
# Within each 16-element page the weights are formed as w^(i+1) * w^-(j+1)
# via three no-bubble running products/sums; two non-consuming bubble uOps at
# each page boundary rescale the carry by w^16 exactly. fp32 range needs
# w^-15 finite: OK for w >= sigmoid(-5.8).
# ---------------------------------------------------------------------------

CURR = AluInp.CURR_ALU_OUT
SWAP = AluInp.CURR_SWAP_OUT
LANE_M = AluInp.PREV_DELAY_3
LANE_R = AluInp.PREV_DELAY_4
PAGE = 16
_BYP = _Stage(AluOp.BYPASS, PREV)


def _build_ema_uops():
    m_key = Bin(AluOp.MULTIPLY, Src0, C1)
    r_key = Bin(AluOp.MULTIPLY, C0, C0)
    p = _Placement(
        pipeline=[
            _Stage(AluOp.MULTIPLY, CURR, C1),     # st0: Inv <- Inv * (1/w)
            _Stage(AluOp.MULTIPLY, Src0, PREV),   # st1: m = u * Inv
            _Stage(AluOp.MULTIPLY, CURR, C0),     # st2: R <- R * w
            _Stage(AluOp.ADD, CURR, LANE_M),      # st3: A <- A + m
            _Stage(AluOp.MULTIPLY, PREV, LANE_R), # st4: out = A * R
            _BYP, _BYP, _BYP,
        ],
        node_stage={},
        lane={Src0: 0, C0: 1, C1: 2, m_key: 3, r_key: 4, One: 5},
        out_sel=OutSel.ALU_OUT,
        accum_stage=None,
        captures=[(2, 3), (3, 4)],
    )
    latch_p = _Placement(
        pipeline=[_BYP] * 8, node_stage={}, lane={Src1: 0},
        out_sel=OutSel.ALU_OUT, accum_stage=None, captures=[],
    )
    states = [
        _State(  # 0: latch-init — park the carry (in1) in st3's swap flop
            placement=latch_p, trigger=COUNT_ONCE, repeat=1,
            consume=(False, True),
            overrides={3: _Stage(AluOp.BYPASS, Src1, Src1, swap=True)},
            write_out=False, next=(1, 0, 0),
        ),
        _State(  # 1: seed — Inv=1, R=1, A=carry
            placement=p, trigger=COUNT_ONCE, repeat=1, consume=(False, False),
            overrides={
                0: _Stage(AluOp.BYPASS, One),
                2: _Stage(AluOp.BYPASS, One),
                3: _Stage(AluOp.BYPASS, SWAP),
            },
            write_out=False, next=(2, 0, 0),
        ),
        _State(  # 2: steady — 1 element/cycle; page wrap -> bubbles
            placement=p,
            trigger=(Trigger.SRC_TENSOR_DONE, Trigger.SUB_DIM_DONE, Trigger.NONE),
            next=(0, 3, 0), repeat=0, consume=(True, False), write_out=True,
        ),
        _State(  # 3: B1 — A <- A * w^PAGE (R held at st2)
            placement=p, trigger=COUNT_ONCE, repeat=1, consume=(False, False),
            overrides={
                0: _Stage(AluOp.BYPASS, CURR), 1: _BYP,
                2: _Stage(AluOp.BYPASS, CURR),
                3: _Stage(AluOp.MULTIPLY, CURR, PREV), 4: _BYP,
            },
            write_out=False, next=(4, 0, 0),
        ),
        _State(  # 4: B2 — reset Inv/R for the new page, hold A
            placement=p, trigger=COUNT_ONCE, repeat=1, consume=(False, False),
            overrides={
                0: _Stage(AluOp.BYPASS, One), 1: _BYP,
                2: _Stage(AluOp.BYPASS, One),
                3: _Stage(AluOp.BYPASS, CURR), 4: _BYP,
            },
            write_out=False, next=(2, 0, 0),
        ),
    ]
    uops = [_assemble(s) for s in states]
    for u in uops:
        u.validate("v3")
    return uops


def _ema_ref(in0, in1, s0, s1, imm2):
    P = in0.shape[0]
    u = in0.astype(np.float64).reshape(P, -1)
    w = np.asarray(s0, np.float64).reshape(P, 1)
    r = np.asarray(in1, np.float64).reshape(P, 1)[:, 0].copy()
    out = np.empty_like(u)
    for t in range(u.shape[1]):
        r = w[:, 0] * r + u[:, t]
        out[:, t] = r
    return out.reshape(in0.shape).astype(np.float32)


class _HandDveOp(DveOp):
    def compile(self, ver):
        if ver != "v3":
            raise ValueError(f"{self.name}: hand-built for v3/TRN2 only")
        cached = dve_ops._COMPILE_CACHE.get((self.name, ver))
        if cached is not None:
            return cached
        spec = DveOpSpec(
            name=self.name,
            opcode=dve_ops.get_dve_sub_opcode(self.name),
            uops=_build_ema_uops(),
            rd1_en=True,
        )
        dve_ops._COMPILE_CACHE[(self.name, ver)] = spec
        return spec


def _register_ema_op() -> DveOp:
    for op in dve_ops.OPS:
        if op.name == "EMA_PAGED_ANT":
            return op
    op = _HandDveOp(
        "EMA_PAGED_ANT",
        Spec(
            body=Bin(AluOp.ADD, Bin(AluOp.MULTIPLY, Src0, C1),
                     Bin(AluOp.MULTIPLY, Src1, C0)),
            reference=_ema_ref,
        ),
        subdim=True,
        uops_sha={},
    )
    dve_ops.OPS.append(op)
    dve_ops.CUSTOM_DVE_SPECS[op.name] = op.spec
    dve_ops._SUB_OPCODE_FOR_NAME[op.name] = (
        dve_ops._CUSTOM_DVE_ROW_BASE + len(dve_ops.OPS) - 1
    )
    assert dve_ops._SUB_OPCODE_FOR_NAME[op.name] < 0x20
    return op


# ---------------------------------------------------------------------------
# Kernel
# ---------------------------------------------------------------------------

B, T, D = 16, 4096, 512
NCORES = 8
BL = B // NCORES   # batches per core
P = 128            # partitions
TCW = 1024         # time chunk per pipeline iteration
ND = D // P        # channel chunks of 128 (4)
NK = TCW // P      # 128-row sub-chunks per time chunk (8)

FP32 = mybir.dt.float32
BF16 = mybir.dt.bfloat16


def build_program(bl: int = BL, t: int = T) -> bacc.Bacc:
    ema = _register_ema_op()
    ntc = t // TCW
    nc = bacc.Bacc(
        "TRN2",
        target_bir_lowering=False,
        debug=False,
        enable_asserts=False,
        num_devices=NCORES,
    )
    x = nc.dram_tensor("x", (bl, t, D), BF16, kind="ExternalInput").ap()
    alpha = nc.dram_tensor("alpha", (1, 1, D), FP32, kind="ExternalInput").ap()
    y = nc.dram_tensor("y", (bl, t, D), BF16, kind="ExternalOutput").ap()

    with tile.TileContext(nc) as tc, ExitStack() as ctx:
        const_pool = ctx.enter_context(tc.tile_pool(name="const", bufs=1))
        xn_pool = ctx.enter_context(tc.tile_pool(name="xn", bufs=3))
        pin_pool = ctx.enter_context(tc.tile_pool(name="pin", bufs=4, space="PSUM"))
        pout_pool = ctx.enter_context(tc.tile_pool(name="pout", bufs=2, space="PSUM"))
        s_pool = ctx.enter_context(tc.tile_pool(name="s", bufs=10))
        y_pool = ctx.enter_context(tc.tile_pool(name="y", bufs=2))
        carry_pool = ctx.enter_context(tc.tile_pool(name="carry", bufs=1))

        ident = const_pool.tile([P, P], BF16)
        make_identity(nc, ident[:])

        # alpha (1,1,512) -> (128, ND): channel d = j*128 + p
        alpha_sb = const_pool.tile([P, ND], FP32)
        nc.sync.dma_start(alpha_sb[:], alpha.rearrange("o u (j p) -> (o u p) j", p=P))
        a_sb = const_pool.tile([P, ND], FP32)   # a = sigmoid(alpha)
        nc.scalar.activation(a_sb[:], alpha_sb[:], mybir.ActivationFunctionType.Sigmoid)
        w_sb = const_pool.tile([P, ND], FP32)   # w = 1 - a = sigmoid(-alpha)
        nc.scalar.activation(
            w_sb[:], alpha_sb[:], mybir.ActivationFunctionType.Sigmoid, scale=-1.0
        )
        inv_w = const_pool.tile([P, ND], FP32)
        nc.vector.reciprocal(inv_w[:], w_sb[:])
        inv_a = const_pool.tile([P, ND], FP32)
        nc.vector.reciprocal(inv_a[:], a_sb[:])

        # diag(a) per channel chunk: ident row p scaled by a[p] (bf16)
        diags = []
        for j in range(ND):
            dg = const_pool.tile([P, P], BF16, tag=f"diag{j}")
            nc.vector.tensor_scalar_mul(dg[:], ident[:], a_sb[:, j : j + 1])
            diags.append(dg)

        inits = carry_pool.tile([P, bl * ND], FP32)

        s_prevs = [[None] * ND for _ in range(bl)]
        for tci in range(ntc):
            for b in range(bl):
                t0 = tci * TCW
                xn = xn_pool.tile([P, NK, D], BF16, tag="xn")
                nsplit = 4 if (tci == 0 and b == 0) else 2
                kq, tq = NK // nsplit, TCW // nsplit
                for hh in range(nsplit):
                    nc.sync.dma_start(
                        xn[:, hh * kq : (hh + 1) * kq, :],
                        x[b, t0 + hh * tq : t0 + (hh + 1) * tq, :].rearrange(
                            "(k p) d -> p k d", p=P
                        ),
                    )

                # Transpose to (d-part, t-free) PSUM, then scan straight out
                # of PSUM with the custom EMA op.
                ss = []
                for j in range(ND):
                    pin = pin_pool.tile([P, TCW], BF16, tag="pin")
                    for k in range(NK):
                        nc.tensor.transpose(
                            pin[:, k * P : (k + 1) * P],
                            xn[:, k, j * P : (j + 1) * P],
                            ident[:],
                        )
                    if tci == 0:
                        # carry r_{-1} = x_0 / a  =>  s_0 = x_0 exactly
                        nc.vector.tensor_scalar_mul(
                            inits[:, b * ND + j : b * ND + j + 1],
                            pin[:, 0:1],
                            inv_a[:, j : j + 1],
                        )
                        carry = inits[:, b * ND + j : b * ND + j + 1]
                    else:
                        carry = s_prevs[b][j][:, TCW - 1 : TCW]
                    s = s_pool.tile([P, TCW], BF16, tag="s", name=f"s{j}_{b}_{tci}")
                    nc.vector._custom_dve(
                        ema,
                        out=s[:].rearrange("p (s n) -> p s n", n=PAGE),
                        in0=pin[:].rearrange("p (s n) -> p s n", n=PAGE),
                        in1=carry,
                        s0=w_sb[:, j : j + 1],
                        s1=inv_w[:, j : j + 1],
                    )
                    ss.append(s)
                s_prevs[b] = ss

                # Transpose back with diag(a): yout = (a * r)^T = s^T.
                yout = y_pool.tile([P, NK, D], BF16, tag="y")
                for m in range(NK // 2):
                    pout = pout_pool.tile([P, 2 * D], FP32, tag="pout")
                    for h in range(2):
                        k = 2 * m + h
                        for j in range(ND):
                            nc.tensor.matmul(
                                pout[:, h * D + j * P : (h * D + (j + 1) * P)],
                                ss[j][:, k * P : (k + 1) * P],
                                diags[j][:],
                            )
                    last = tci == ntc - 1 and b == bl - 1
                    if last and m % 2 == 1:
                        nc.vector.tensor_copy(yout[:, 2 * m : 2 * m + 2, :], pout[:])
                    else:
                        nc.scalar.copy(yout[:, 2 * m : 2 * m + 2, :], pout[:])
                half = TCW // 2
                for hh in range(2):
                    nc.gpsimd.dma_start(
                        y[b, t0 + hh * half : t0 + (hh + 1) * half, :].rearrange(
                            "(k p) d -> p k d", p=P
                        ),
                        yout[:, hh * (NK // 2) : (hh + 1) * (NK // 2), :],
                    )

    nc.compile()
    return nc


_prog = None


def kernel(x, alpha):
    global _prog
    if _prog is None:
        _prog = build_program()
    x = np.asarray(x)
    alpha = np.ascontiguousarray(np.asarray(alpha, dtype=np.float32))
    assert x.shape == (B, T, D) and alpha.shape == (1, 1, D)
    xb = np.ascontiguousarray(x.astype(ml_dtypes.bfloat16))
    in_maps = [
        {"x": np.ascontiguousarray(xb[i * BL : (i + 1) * BL]), "alpha": alpha}
        for i in range(NCORES)
    ]
    res = run_bass_kernel_spmd(_prog, in_maps, core_ids=list(range(NCORES)))
    out = np.concatenate([r["y"] for r in res.results], axis=0)
    return np.ascontiguousarray(out.astype(np.float32))
